# revision 2
# baseline (speedup 1.0000x reference)
"""Cost-volume concatenation kernel for Trainium2 (8 NeuronCores) — v4.

Reference computation:
    out[b, c,    d, h, x] = left [b, c, h, x]          if 0 <= x - disp_d < W else 0
    out[b, C+c,  d, h, x] = right[b, c, h, x - disp_d] if 0 <= x - disp_d < W else 0
with disp_d = d - 112 for d in [0, 128), shapes left/right [1, 32, 128, 256] f32,
output [1, 64, 128, 128, 256] f32 (1 GiB).  Pure data movement -> DMA-only kernel.

Sharding: H is split 16 rows per core (identical SPMD program per core).
The device output is d-major [D, 2C, HS, W]; the host transposes (c, d) while
gathering shards.

Measured HW facts this design is built on (probed on THIS platform, see
probe_ps.py / probe4.py / probe5.py):
  * Store throughput on one core is limited by per-DMA concurrency, not by
    descriptor size or HBM: 2 MiB stores stream at ~113 GB/s/core (same with
    1 core active -> per-core cap), while the same bytes as 512 KiB
    fully-contiguous-dst stores run ~288 GB/s/core (466 us vs 1187 us per
    128 MiB).  Dual-ring (sync+scalar) and gpsimd rings gain little.
  * DVE window copies are ~free (32 x 2 MiB/rep unmeasurable vs noise).
  * DMA completions across in-flight transfers are NOT ordered: every wait
    on a DMA-completion semaphore must be on a per-resource semaphore at its
    current in-flight maximum (CoreSim's race detector enforces this).

Design:
  * Right half: a resident "rt" tile holds 4 pre-shifted replicas of the
    zero-padded right rows: partition p = 32q + c holds channel c's 16 rows
    of R_q[y] = P[y + 3 - q] (y in [0,380)), P = [15 zeros | right | 113
    zeros].  One 128-partition DVE tensor_copy per 4-disparity group j
    (window [j, j+256)) assembles quadrant q = disparity d = 124 - j + q
    into an asm ring buffer (NASM=4); each partition holds its (d, c) output
    block 16 KiB contiguous.  The rpad zeros provide masking for free.
  * Left half: three c-block tiles hold left replicated in 4 quadrants;
    quadrant q serves disparity d0 + q.  The scalar engine extends the
    per-quadrant zero margins in place (activation copy x0.0) between uses;
    two tiles ping-pong over the 28 negative-disparity groups (j = 16..124,
    d descending), a third serves the 4 positive groups (j = 12, 8, 4, 0).
  * ALL stores are per-(d, c-slice) DMAs with fully contiguous dst
    (CSPLIT pieces per half-d-block) issued on the single sync ring in
    group order; loads are split the same way.
"""

import sys

sys.path.insert(0, "/opt/trn_rl_repo")

import numpy as np

import concourse.bass as bass
import concourse.mybir as mybir
from concourse.bass_utils import run_bass_kernel_spmd

F32 = mybir.dt.float32
N_CORES = 8
B, C, H, W = 1, 32, 128, 256
HS = H // N_CORES          # 16 rows of H per core
D = 128                    # disparities; disp = d - 112
RW = 380                   # rsrc row width
NASM = 4                   # asm ring buffers
CSPLIT = 1                 # c-slices per (d, half) store: 1 -> 512 KiB stores
CS = 32 // CSPLIT          # channels per store
SPG = 4 * CSPLIT           # stores per group per half

# group schedule: j in neg (d<=111, margins grow) then pos (prefix grows)
NEG_JS = list(range(16, 125, 4))          # 28 groups, d desc 111..0
POS_JS = [12, 8, 4, 0]                    # 4 groups, d asc 112..127
ALL_JS = NEG_JS + POS_JS                  # 32 groups; group k -> d0 = 124 - j

_PROGRAMS = {}


def _build_program(repeat=1):
    """Build the SPMD program. `repeat` re-runs the full pass N times on the
    same output (used by the test harness for differential HW timing)."""
    nc = bass.Bass()
    lpad4 = nc.declare_dram_parameter("lpad4", [4 * C * HS, W], F32, isOutput=False)
    rsrc = nc.declare_dram_parameter("rsrc", [128, HS * RW], F32, isOutput=False)
    out = nc.declare_dram_parameter("out", [D, 2 * C, HS, W], F32, isOutput=True)

    with (
        nc.sbuf_tensor([128, HS * W], F32) as t0,   # left c-block tiles
        nc.sbuf_tensor([128, HS * W], F32) as t1,
        nc.sbuf_tensor([128, HS * W], F32) as tcp,
        nc.sbuf_tensor([128, HS * RW], F32) as rt,  # resident pre-shifted right
        nc.sbuf_tensor([128, NASM * HS * W], F32) as am,  # asm ring
        nc.semaphore("rs_sem") as rs_sem,           # rsrc load done
        nc.semaphore("la_sem") as la_sem,           # per-tile load done
        nc.semaphore("lb_sem") as lb_sem,
        nc.semaphore("lc_sem") as lc_sem,
        nc.semaphore("ms_sem") as ms_sem,           # scalar memset batches done
        nc.semaphore("ar_sem") as ar_sem,           # asm buffers ready (DVE)
        nc.semaphore("af0_sem") as af0_sem,         # per-asm-buffer stores done
        nc.semaphore("af1_sem") as af1_sem,
        nc.semaphore("af2_sem") as af2_sem,
        nc.semaphore("af3_sem") as af3_sem,
        nc.semaphore("ta_sem") as ta_sem,           # per-left-tile stores done
        nc.semaphore("tb_sem") as tb_sem,
        nc.semaphore("tc_sem") as tc_sem,
        nc.Block() as block,
    ):
        ltiles = [t0, t1, tcp]
        tile_sems = [ta_sem, tb_sem, tc_sem]
        load_sems = [la_sem, lb_sem, lc_sem]
        af_sems = [af0_sem, af1_sem, af2_sem, af3_sem]
        rt3 = rt[:, :].rearrange("p (h y) -> p h y", h=HS)
        t3 = [t[:, :].rearrange("p (h x) -> p h x", h=HS) for t in ltiles]
        am4 = am[:, :].rearrange("p (n h x) -> p n h x", n=NASM, h=HS)

        # group k: k<28 -> neg, tile k%2; k>=28 -> pos, tile 2
        def group_tile(k):
            return (k % 2) if k < 28 else 2

        # per-tile cumulative (within-rep) use index, 1-based, for group k
        tile_use = {}
        cnt = [0, 0, 0]
        for k in range(32):
            t = group_tile(k)
            cnt[t] += 1
            tile_use[k] = cnt[t]
        USES = list(cnt)  # per-rep store-group count per tile: [14, 14, 4]
        BUF_USES = [len(range(b, 32, NASM)) for b in range(NASM)]  # 8 each
        SINC = 16 * SPG   # sem increments per store-group (SPG DMAs x 16)

        @block.sync
        def _(sync):
            for rep in range(repeat):
                if rep == 0:
                    for q in range(4):
                        sync.dma_start(
                            out=rt[32 * q : 32 * (q + 1), :],
                            in_=rsrc[32 * q : 32 * (q + 1), :],
                        ).then_inc(rs_sem, 16)
                for t in range(3):
                    if rep > 0:
                        sync.wait_ge(tile_sems[t], SINC * USES[t] * rep)
                    for q in range(4):
                        sync.dma_start(
                            out=ltiles[t][32 * q : 32 * (q + 1), :],
                            in_=lpad4[512 * q : 512 * (q + 1), :],
                        ).then_inc(load_sems[t], 16)

                for k, j in enumerate(ALL_JS):
                    d0 = 124 - j
                    b = k % NASM
                    # right stores: asm buffer b, assembled by the DVE
                    sync.wait_ge(ar_sem, 32 * rep + k + 1)
                    for q in range(4):
                        for i in range(CSPLIT):
                            c0 = CS * i
                            p0 = 32 * q + c0
                            sync.dma_start(
                                out=out[d0 + q, C + c0 : C + c0 + CS, :, :],
                                in_=am[
                                    p0 : p0 + CS,
                                    b * HS * W : (b + 1) * HS * W,
                                ],
                            ).then_inc(af_sems[b], 16)
                    # left stores: margins prepared by the scalar engine
                    sync.wait_ge(ms_sem, 32 * rep + k + 1)
                    t = group_tile(k)
                    for q in range(4):
                        for i in range(CSPLIT):
                            c0 = CS * i
                            p0 = 32 * q + c0
                            sync.dma_start(
                                out=out[d0 + q, c0 : c0 + CS, :, :],
                                in_=ltiles[t][p0 : p0 + CS, :],
                            ).then_inc(tile_sems[t], 16)
            # final drain
            for b in range(NASM):
                sync.wait_ge(af_sems[b], SINC * BUF_USES[b] * repeat)
            for t in range(3):
                sync.wait_ge(tile_sems[t], SINC * USES[t] * repeat)

        @block.vector
        def _(vector):
            vector.wait_ge(rs_sem, 64)  # rt resident
            for rep in range(repeat):
                for k, j in enumerate(ALL_JS):
                    b = k % NASM
                    # all previous uses of this buffer must be fully stored;
                    # its next consumer is gated on ar (this op), so the wait
                    # value is the semaphore's current in-flight maximum.
                    prev_uses = BUF_USES[b] * rep + k // NASM
                    if prev_uses > 0:
                        vector.wait_ge(af_sems[b], SINC * prev_uses)
                    vector.tensor_copy(
                        out=am4[:, b, :, :], in_=rt3[:, :, j : j + W]
                    ).then_inc(ar_sem, 1)

        @block.scalar
        def _(scalar):
            # left margins: group k with d0 = 124-j covers d = d0+q.
            #   d < 112: valid x in [0, 144+d)  -> zero [144+d, 256)
            #   d >= 112: valid x in [d-112, W) -> zero [0, d-112)
            def margin(k):
                j = ALL_JS[k]
                d0 = 124 - j
                spans = []
                for q in range(4):
                    d = d0 + q
                    if d < 112:
                        lo = 144 + d
                        # previous use of this tile (k-2) had lo' = lo + 8;
                        # first use zeroes the whole margin
                        hi = 256 if k < 2 else min(256, lo + 8)
                        if hi > lo:
                            spans.append((q, lo, hi))
                    else:
                        hi = d - 112
                        lo = 0 if k == 28 else hi - 4
                        if hi > lo:
                            spans.append((q, lo, hi))
                return spans

            for rep in range(repeat):
                for k in range(32):
                    t = group_tile(k)
                    # this rep's load of tile t must have landed ...
                    scalar.wait_ge(load_sems[t], 64 * (rep + 1))
                    # ... and the tile's previous store-group must be done
                    # (its next store is gated on ms from THIS batch, so the
                    # wait value is the semaphore's in-flight maximum).
                    prev_use = tile_use[k] - 1 + USES[t] * rep
                    if prev_use > 0:
                        scalar.wait_ge(tile_sems[t], SINC * prev_use)
                    spans = margin(k)
                    assert spans, f"group {k} has no margin spans"
                    ops = []
                    for q, lo, hi in spans:
                        ops.append(
                            scalar.mul(
                                t3[t][32 * q : 32 * (q + 1), :, lo:hi],
                                t3[t][32 * q : 32 * (q + 1), :, lo:hi],
                                0.0,
                            )
                        )
                    ops[-1].then_inc(ms_sem, 1)

    return nc


def _get_program(repeat=1):
    if repeat not in _PROGRAMS:
        _PROGRAMS[repeat] = _build_program(repeat)
    return _PROGRAMS[repeat]


def make_in_maps(left, right):
    """Host-side sharding: slice H into per-core row blocks and build the
    padded input tensors."""
    in_maps = []
    for i in range(N_CORES):
        h0 = i * HS
        lrows = np.ascontiguousarray(
            left[0, :, h0 : h0 + HS, :]
        ).reshape(C * HS, W)
        lpad4 = np.ascontiguousarray(
            np.broadcast_to(lrows.reshape(1, C * HS, W), (4, C * HS, W))
        ).reshape(4 * C * HS, W)
        # rpad row P: [15 zeros | right row | 113 zeros] (384 cols);
        # rsrc partition 32q+c = channel c's 16 rows of P[y+3-q], y in [0,380)
        rp = np.zeros((C, HS, 384), dtype=np.float32)
        rp[:, :, 15 : 15 + W] = right[0, :, h0 : h0 + HS, :]
        rsrc = np.empty((4, C, HS, RW), dtype=np.float32)
        for q in range(4):
            rsrc[q] = rp[:, :, 3 - q : 3 - q + RW]
        in_maps.append(
            {"lpad4": lpad4, "rsrc": np.ascontiguousarray(rsrc.reshape(128, HS * RW))}
        )
    return in_maps


def kernel(left, right):
    left = np.asarray(left, dtype=np.float32)
    right = np.asarray(right, dtype=np.float32)
    nc = _get_program()
    in_maps = make_in_maps(left, right)
    res = run_bass_kernel_spmd(nc, in_maps, list(range(N_CORES))).results
    outf = np.empty((B, 2 * C, D, H, W), dtype=np.float32)
    for i in range(N_CORES):
        # device shard is d-major [D, 2C, HS, W] -> transpose to (c, d)
        outf[0, :, :, i * HS : (i + 1) * HS, :] = res[i]["out"].transpose(1, 0, 2, 3)
    return outf
